# Initial kernel scaffold
#
"""Trainium2 Bass kernel for CustomCosineSimCodebook (vq_codebook).

Problem: x [8, 2048, 512] f32 tokens, embed [1, 8192, 512] f32 unit-norm
codebook. Outputs (matching the reference nn.Module):
  quantize [8, 2048, 512] f32  -- codebook rows gathered at argmax indices
  ind      [8, 2048]      int32 -- argmax over codes of x @ embed^T
  dist     [1, 8, 2048, 8192] f32 -- full similarity scores

Sharding: data-parallel over the b*n token axis; core k takes batch row k
(2048 tokens). The codebook is replicated.

Numerics: the scores must be fp32-grade (the argmax feeds an exact gather;
one flipped index is a large error). fp32 matmul on the PE costs 4x. Instead
each operand is split hi/lo into two fp16 halves (lo pre-scaled by 2^11 so
it stays in fp16 normal range), and
    dist = x_hi @ e_hi + 2^-11 * (x_lo' @ e_hi + x_hi @ e_lo')
runs as 3 fp16 passes at full PE rate, accumulated in fp32 PSUM (A and B
accumulate in separate banks; one fused DVE op merges them). Dropped
lo*lo term ~2^-23 relative => score error ~1e-6: fp32-grade.

Argmax: DVE MAX/MAX_INDEX over each token's full 8192-score row in SBUF
(first-match on ties, same as jnp.argmax). Quantize: indirect-DMA gather of
fp32 codebook rows (bit-exact vs reference given equal indices).
"""

import numpy as np

import concourse.bass as bass
import concourse.mybir as mybir
import concourse.tile as tile

B, N, D, C = 8, 2048, 512, 8192
NCORES = 8
TOK = B * N // NCORES        # 2048 tokens per core
P = 128                      # partitions
KC = D // P                  # 4 contraction chunks
NCH = C // 512               # 16 n-chunks of 512 codes
GROUP = 4                    # n-chunks in flight (each uses 2 PSUM banks)
SCALE = 2.0 ** 11
F16_MIN_NORMAL = 6.104e-5

LAST_RESULT = None           # BassKernelResults of the most recent run


def build(n_mtiles=TOK // P):
    """Build the per-core Bass module (SPMD: same NEFF on all cores)."""
    nc = bass.Bass()
    tok = n_mtiles * P

    xh_d = nc.dram_tensor("xh", [D, tok], mybir.dt.float16, kind="ExternalInput")
    xl_d = nc.dram_tensor("xl", [D, tok], mybir.dt.float16, kind="ExternalInput")
    eh_d = nc.dram_tensor("eh", [D, C], mybir.dt.float16, kind="ExternalInput")
    el_d = nc.dram_tensor("el", [D, C], mybir.dt.float16, kind="ExternalInput")
    emb_d = nc.dram_tensor("emb", [C, D], mybir.dt.float32, kind="ExternalInput")

    dist_d = nc.dram_tensor("dist", [tok, C], mybir.dt.float32, kind="ExternalOutput")
    ind_d = nc.dram_tensor("ind", [tok, 1], mybir.dt.uint32, kind="ExternalOutput")
    quant_d = nc.dram_tensor("quant", [tok, D], mybir.dt.float32, kind="ExternalOutput")

    with tile.TileContext(nc) as tc:
        with (
            tc.tile_pool(name="codebook", bufs=1) as cpool,
            tc.tile_pool(name="xtiles", bufs=2) as xpool,
            tc.tile_pool(name="distrow", bufs=1) as dpool,
            tc.tile_pool(name="outs", bufs=2) as opool,
            tc.tile_pool(name="psum", bufs=1, space="PSUM") as ppool,
        ):
            # Codebook resident in SBUF: 8 tiles [128, 8192] fp16 (128 KB/part).
            # Loads split at 2048-col boundaries so group g of m-tile 0 only
            # waits for its own column range.
            eh_t, el_t = [], []
            for kc in range(KC):
                rs = slice(kc * P, (kc + 1) * P)
                th = cpool.tile([P, C], mybir.dt.float16, name=f"ehs{kc}")
                tl = cpool.tile([P, C], mybir.dt.float16, name=f"els{kc}")
                for j in range(0, C, 512 * GROUP):
                    cs = slice(j, j + 512 * GROUP)
                    nc.sync.dma_start(th[:, cs], eh_d[rs, cs])
                    nc.sync.dma_start(tl[:, cs], el_d[rs, cs])
                eh_t.append(th)
                el_t.append(tl)

            for m in range(n_mtiles):
                ms = slice(m * P, (m + 1) * P)
                xh_t, xl_t = [], []
                for kc in range(KC):
                    rs = slice(kc * P, (kc + 1) * P)
                    th = xpool.tile([P, P], mybir.dt.float16, tag=f"xh{kc}")
                    tl = xpool.tile([P, P], mybir.dt.float16, tag=f"xl{kc}")
                    nc.sync.dma_start(th[:], xh_d[rs, ms])
                    nc.sync.dma_start(tl[:], xl_d[rs, ms])
                    xh_t.append(th)
                    xl_t.append(tl)

                dist_row = dpool.tile([P, C], mybir.dt.float32, tag="dist_row")

                for g in range(NCH // GROUP):
                    pa = [ppool.tile([P, 512], mybir.dt.float32, tag=f"pa{i}")
                          for i in range(GROUP)]
                    pb = [ppool.tile([P, 512], mybir.dt.float32, tag=f"pb{i}")
                          for i in range(GROUP)]
                    csl = [slice((g * GROUP + i) * 512, (g * GROUP + i + 1) * 512)
                           for i in range(GROUP)]
                    # One x_hi weight load serves 2*GROUP matmuls (A and B).
                    for kc in range(KC):
                        for i in range(GROUP):
                            nc.tensor.matmul(pa[i][:], xh_t[kc][:], eh_t[kc][:, csl[i]],
                                             start=(kc == 0), stop=(kc == KC - 1))
                        for i in range(GROUP):
                            nc.tensor.matmul(pb[i][:], xh_t[kc][:], el_t[kc][:, csl[i]],
                                             start=(kc == 0), stop=False)
                    for kc in range(KC):
                        for i in range(GROUP):
                            nc.tensor.matmul(pb[i][:], xl_t[kc][:], eh_t[kc][:, csl[i]],
                                             start=False, stop=(kc == KC - 1))
                    for i in range(GROUP):
                        # dist = A + 2^-11 * B, PSUM -> SBUF, one fused DVE op
                        nc.vector.scalar_tensor_tensor(
                            dist_row[:, csl[i]], pb[i][:], 1.0 / SCALE, pa[i][:],
                            mybir.AluOpType.mult, mybir.AluOpType.add)
                        nc.sync.dma_start(dist_d[ms, csl[i]], dist_row[:, csl[i]])

                max8 = opool.tile([P, 8], mybir.dt.float32, tag="max8")
                idx8 = opool.tile([P, 8], mybir.dt.uint32, tag="idx8")
                nc.vector.max(max8[:], dist_row[:])
                nc.vector.max_index(idx8[:], max8[:], dist_row[:])

                qt = opool.tile([P, D], mybir.dt.float32, tag="qt")
                nc.gpsimd.indirect_dma_start(
                    out=qt[:], out_offset=None, in_=emb_d[:, :],
                    in_offset=bass.IndirectOffsetOnAxis(ap=idx8[:, :1], axis=0))
                nc.sync.dma_start(quant_d[ms, :], qt[:])
                nc.sync.dma_start(ind_d[ms, :], idx8[:, :1])
    return nc


def split_fp16(a):
    """a (f32) -> (hi, lo) fp16 with lo pre-scaled by 2^11; no denormals.
    a ~= hi + lo * 2^-11 to ~2^-23 relative."""
    hi = a.astype(np.float16)
    hi = np.where(np.abs(hi) < F16_MIN_NORMAL, np.float16(0), hi)
    lo = ((a - hi.astype(np.float32)) * np.float32(SCALE)).astype(np.float16)
    lo = np.where(np.abs(lo) < F16_MIN_NORMAL, np.float16(0), lo)
    return hi, lo


_nc_cache = {}


def _get_nc(n_mtiles):
    if n_mtiles not in _nc_cache:
        _nc_cache[n_mtiles] = build(n_mtiles)
    return _nc_cache[n_mtiles]


def kernel(x, embed, _trace=False):
    global LAST_RESULT
    from concourse.bass_utils import run_bass_kernel_spmd

    x = np.ascontiguousarray(np.asarray(x, dtype=np.float32))
    e = np.ascontiguousarray(np.asarray(embed, dtype=np.float32)[0])  # [C, D]

    eT = np.ascontiguousarray(e.T)                  # [D, C]
    eh, el = split_fp16(eT)
    in_maps = []
    for k in range(NCORES):
        xT = np.ascontiguousarray(x[k].T)           # [D, 2048]
        xh, xl = split_fp16(xT)
        in_maps.append({"xh": xh, "xl": xl, "eh": eh, "el": el, "emb": e})

    nc = _get_nc(TOK // P)
    LAST_RESULT = run_bass_kernel_spmd(
        nc, in_maps, core_ids=list(range(NCORES)), trace=_trace)
    results = LAST_RESULT.results

    quant = np.stack([r["quant"] for r in results])                  # [8,2048,512]
    ind = np.stack([r["ind"][:, 0].astype(np.int32) for r in results])  # [8,2048]
    dist = np.stack([r["dist"] for r in results])[None]              # [1,8,2048,8192]
    return quant, ind, dist


# revision 15
# speedup vs baseline: 1.1853x; 1.1853x over previous
"""Trainium2 Bass kernel for CustomCosineSimCodebook (vq_codebook).

Problem: x [8, 2048, 512] f32 tokens, embed [1, 8192, 512] f32 unit-norm
codebook. Outputs (matching the reference nn.Module):
  quantize [8, 2048, 512] f32  -- codebook rows gathered at argmax indices
  ind      [8, 2048]      int32 -- argmax over codes of x @ embed^T
  dist     [1, 8, 2048, 8192] f32 -- full similarity scores

Sharding: data-parallel over the b*n token axis; core k takes batch row k
(2048 tokens). The codebook is replicated.

Numerics: the scores must be fp32-grade (the argmax feeds an exact gather;
one flipped index is a large error). fp32 matmul on the PE costs 4x. Instead
each operand is split hi/lo into two fp16 halves (lo parts pre-scaled by
2^11 so they stay in fp16 normal range, and x_hi additionally provided
pre-scaled by 2^11 as xh_s), and
    2^11 * dist = xh_s @ e_hi + x_lo' @ e_hi + x_hi @ e_lo'
runs as 3 fp16 passes at full PE rate, all accumulating into ONE fp32 PSUM
bank at 2^11 scale; a single scalar-engine Copy with scale=2^-11 (exact)
writes each chunk to SBUF. Dropped lo*lo term ~2^-23 relative => score
error ~1e-6: fp32-grade.

Argmax: DVE MAX/MAX_INDEX over each token's full 8192-score row in SBUF
(first-match on ties, same as jnp.argmax). Quantize: indirect-DMA gather of
fp32 codebook rows (bit-exact vs reference given equal indices).
"""

import numpy as np

import concourse.bass as bass
import concourse.mybir as mybir
import concourse.tile as tile

B, N, D, C = 8, 2048, 512, 8192
NCORES = 8
TOK = B * N // NCORES        # 2048 tokens per core
P = 128                      # partitions
KC = D // P                  # 4 contraction chunks
NCH = C // 512               # 16 n-chunks of 512 codes
GROUP = 8                    # n-chunks in flight (one PSUM bank each)
SCALE = 2.0 ** 11
F16_MIN_NORMAL = 6.104e-5

LAST_RESULT = None           # BassKernelResults of the most recent run


def build(n_mtiles=TOK // P, repeats=1):
    """Build the per-core Bass module (SPMD: same NEFF on all cores).
    repeats>1 re-runs the whole compute body (idempotent outputs) so test
    harnesses can isolate HW exec time by differencing wall times."""
    nc = bass.Bass()
    tok = n_mtiles * P

    xh_d = nc.dram_tensor("xh", [D, tok], mybir.dt.float16, kind="ExternalInput")
    xs_d = nc.dram_tensor("xs", [D, tok], mybir.dt.float16, kind="ExternalInput")
    xl_d = nc.dram_tensor("xl", [D, tok], mybir.dt.float16, kind="ExternalInput")
    eh_d = nc.dram_tensor("eh", [D, C], mybir.dt.float16, kind="ExternalInput")
    el_d = nc.dram_tensor("el", [D, C], mybir.dt.float16, kind="ExternalInput")
    emb_d = nc.dram_tensor("emb", [C, D], mybir.dt.float32, kind="ExternalInput")

    dist_d = nc.dram_tensor("dist", [tok, C], mybir.dt.float32, kind="ExternalOutput")
    ind_d = nc.dram_tensor("ind", [tok, 1], mybir.dt.uint32, kind="ExternalOutput")
    quant_d = nc.dram_tensor("quant", [tok, D], mybir.dt.float32, kind="ExternalOutput")

    with tile.TileContext(nc) as tc:
        with (
            tc.tile_pool(name="codebook", bufs=1) as cpool,
            tc.tile_pool(name="xtiles", bufs=2) as xpool,
            tc.tile_pool(name="distrow", bufs=1) as dpool,
            tc.tile_pool(name="outs", bufs=2) as opool,
            tc.tile_pool(name="psum", bufs=1, space="PSUM") as ppool,
        ):
            # Codebook resident in SBUF: 8 tiles [128, 8192] fp16 (128 KB/part).
            # Loads split at 2048-col boundaries so group g of m-tile 0 only
            # waits for its own column range.
            eh_t, el_t = [], []
            for kc in range(KC):
                rs = slice(kc * P, (kc + 1) * P)
                th = cpool.tile([P, C], mybir.dt.float16, name=f"ehs{kc}")
                tl = cpool.tile([P, C], mybir.dt.float16, name=f"els{kc}")
                for j in range(0, C, 512 * GROUP):
                    cs = slice(j, j + 512 * GROUP)
                    nc.sync.dma_start(th[:, cs], eh_d[rs, cs])
                    nc.sync.dma_start(tl[:, cs], el_d[rs, cs])
                eh_t.append(th)
                el_t.append(tl)

            for rep, m in ((r, mm) for r in range(repeats)
                           for mm in range(n_mtiles)):
                ms = slice(m * P, (m + 1) * P)
                xh_t, xs_t, xl_t = [], [], []
                for kc in range(KC):
                    rs = slice(kc * P, (kc + 1) * P)
                    th = xpool.tile([P, P], mybir.dt.float16, tag=f"xh{kc}")
                    ts_ = xpool.tile([P, P], mybir.dt.float16, tag=f"xs{kc}")
                    tl = xpool.tile([P, P], mybir.dt.float16, tag=f"xl{kc}")
                    nc.sync.dma_start(th[:], xh_d[rs, ms])
                    nc.sync.dma_start(ts_[:], xs_d[rs, ms])
                    nc.sync.dma_start(tl[:], xl_d[rs, ms])
                    xh_t.append(th)
                    xs_t.append(ts_)
                    xl_t.append(tl)

                dist_row = dpool.tile([P, C], mybir.dt.float32, tag="dist_row")

                for g in range(NCH // GROUP):
                    pp = [ppool.tile([P, 512], mybir.dt.float32, tag=f"pp{i}",
                                     name=f"pp{rep}_{m}_{g}_{i}")
                          for i in range(GROUP)]
                    csl = [slice((g * GROUP + i) * 512, (g * GROUP + i + 1) * 512)
                           for i in range(GROUP)]
                    # 3 fp16 passes, one PSUM bank per chunk, at 2^11 scale.
                    # Each weight load feeds GROUP matmuls.
                    for kc in range(KC):
                        for i in range(GROUP):
                            nc.tensor.matmul(pp[i][:], xs_t[kc][:], eh_t[kc][:, csl[i]],
                                             start=(kc == 0), stop=False)
                    for kc in range(KC):
                        for i in range(GROUP):
                            nc.tensor.matmul(pp[i][:], xl_t[kc][:], eh_t[kc][:, csl[i]],
                                             start=False, stop=False)
                    for kc in range(KC):
                        for i in range(GROUP):
                            nc.tensor.matmul(pp[i][:], xh_t[kc][:], el_t[kc][:, csl[i]],
                                             start=False, stop=(kc == KC - 1))
                    for i in range(GROUP):
                        # dist = psum * 2^-11 (exact power-of-2 scale)
                        nc.scalar.activation(dist_row[:, csl[i]], pp[i][:],
                                             mybir.ActivationFunctionType.Copy,
                                             scale=1.0 / SCALE)

                # One big DMA for the whole row: fewer sems (wait-slot limits)
                # and better DMA efficiency than 16 chunked stores.
                nc.sync.dma_start(dist_d[ms, :], dist_row[:])

                max8 = opool.tile([P, 8], mybir.dt.float32, tag="max8")
                idx8 = opool.tile([P, 8], mybir.dt.uint32, tag="idx8")
                nc.vector.max(max8[:], dist_row[:])
                nc.vector.max_index(idx8[:], max8[:], dist_row[:])

                qt = opool.tile([P, D], mybir.dt.float32, tag="qt")
                nc.gpsimd.indirect_dma_start(
                    out=qt[:], out_offset=None, in_=emb_d[:, :],
                    in_offset=bass.IndirectOffsetOnAxis(ap=idx8[:, :1], axis=0))
                nc.sync.dma_start(quant_d[ms, :], qt[:])
                nc.sync.dma_start(ind_d[ms, :], idx8[:, :1])
    return nc


def fix_sync_waits(nc, cap=1):
    """Walrus (this container's version) rejects instructions whose ISA struct
    carries more sync-wait commands than it has slots (DMA/STT: one). Tile
    emits up to ~4. Hoist excess waits onto InstNoOp's inserted immediately
    before the offender on the same engine: the sequencer executes stream-
    order, so blocking on the nop enforces a superset of the original
    ordering. Safe because Tile waits only reference sems incremented by
    other procs / earlier stream positions."""
    skip = {"InstEventSemaphore", "InstISA", "InstCall",
            "InstUnconditionalBranch", "InstCompareAndBranch"}
    n_id = 0
    for fn in nc.m.functions:
        for blk in fn.blocks:
            il = blk.instructions
            i = 0
            while i < len(il):
                inst = il[i]
                tn = type(inst).__name__
                si = inst.sync_info
                if (tn not in skip and si is not None
                        and len(si.on_wait) > cap):
                    waits = list(si.on_wait)
                    excess, keep = waits[:-cap], waits[-cap:]
                    inst.sync_info = mybir.SyncInfo(
                        on_wait=keep, on_update=list(si.on_update))
                    for w in excess:
                        nop = mybir.InstNoOp(
                            name=f"waitnop-{n_id}", engine=inst.engine,
                            ins=[], outs=[])
                        n_id += 1
                        nop.sync_info = mybir.SyncInfo(on_wait=[w], on_update=[])
                        il.insert(i, nop)
                        i += 1
                i += 1
    return nc


def split_fp16(a):
    """a (f32) -> (hi, lo) fp16 with lo pre-scaled by 2^11; no denormals.
    a ~= hi + lo * 2^-11 to ~2^-23 relative."""
    hi = a.astype(np.float16)
    hi = np.where(np.abs(hi) < F16_MIN_NORMAL, np.float16(0), hi)
    lo = ((a - hi.astype(np.float32)) * np.float32(SCALE)).astype(np.float16)
    lo = np.where(np.abs(lo) < F16_MIN_NORMAL, np.float16(0), lo)
    return hi, lo


_nc_cache = {}


def _get_nc(n_mtiles):
    if n_mtiles not in _nc_cache:
        _nc_cache[n_mtiles] = fix_sync_waits(build(n_mtiles))
    return _nc_cache[n_mtiles]


def kernel(x, embed, _trace=False):
    global LAST_RESULT
    from concourse.bass_utils import run_bass_kernel_spmd

    x = np.ascontiguousarray(np.asarray(x, dtype=np.float32))
    e = np.ascontiguousarray(np.asarray(embed, dtype=np.float32)[0])  # [C, D]

    eT = np.ascontiguousarray(e.T)                  # [D, C]
    eh, el = split_fp16(eT)
    in_maps = []
    for k in range(NCORES):
        xT = np.ascontiguousarray(x[k].T)           # [D, 2048]
        xh, xl = split_fp16(xT)
        xs = xh * np.float16(SCALE)                 # exact: power-of-2 scale
        in_maps.append({"xh": xh, "xs": xs, "xl": xl,
                        "eh": eh, "el": el, "emb": e})

    nc = _get_nc(TOK // P)
    LAST_RESULT = run_bass_kernel_spmd(
        nc, in_maps, core_ids=list(range(NCORES)), trace=_trace)
    results = LAST_RESULT.results

    quant = np.stack([r["quant"] for r in results])                  # [8,2048,512]
    ind = np.stack([r["ind"][:, 0].astype(np.int32) for r in results])  # [8,2048]
    dist = np.stack([r["dist"] for r in results])[None]              # [1,8,2048,8192]
    return quant, ind, dist
